# revision 45
# baseline (speedup 1.0000x reference)
"""Trainium2 Bass kernel for nn_Net2_54494545051831 (LocallyConnected2d(7x7)
-> bias -> ReLU -> Linear(28392 -> 10)), on 8 NeuronCores.

Distribution: by output location. Each core owns 3 full output rows
(h = 3c .. 3c+2) plus a 6-7 wide piece of rows 24/25 -> 84/85 locations.
Weights / bias / lw are sharded by location (nothing replicated); each core
computes a partial [10, B] of the final linear layer, summed on host.

Per-core compute ("band" layout): for each owned output row, x is reordered
host-side so the contraction rows of location (h, w) sit at band partitions
[22w, 22w+153): band row 22*w' + j = x[:, j//7, h + j%7, w'] for j < 21,
1.0 at j == 21 (bias folds into the weights), 0 above.

Locations are processed in GROUPS of 3 (42*3 = 126 output channels + 2 zero
columns = full 128-wide stationary operand). A group's 3 windows span
[22w0, 22w0+197) -> 2-3 aligned 128-row band tiles; one full-width matmul
per (group, tile, n-chunk) with zero weights on rows outside each location's
window. ReLU alternates between Vector and Scalar engines; the linear layer
contracts each group's relu'd [128, 512] tile with a [128, 10] per-group lw
slice, accumulated in 4 PSUM column-tile slices (concurrent col-tiled
matmuls) that are summed on host. bf16 matmuls with fp32 accumulation.

Schedule: the DMA HW queues are line-rate limited (~10ns per partition
line), so every transfer uses >=1.75KB lines: band tiles ride the sync
queue one [128,1024] tile at a time in consumption order, weights ride the
scalar queue as contiguous >=7-chunk pieces (each its own DRAM tensor).
The PE must never idle once active (the HW activity manager needs ~4us of
continuous activity to reach full clock and resets on a long stall), so 11
warm-up matmuls bridge kernel entry to the arrival of the first band tiles
(~12.4us: data + per-queue completion-semaphore lag that grows ~0.4us per
outstanding DMA). Row 0 then opens with a "runway": groups 0-2's
accumulation chains interleaved chunk-major (all tile-0 matmuls, then
tile-1, then tile-2) so the PE never outruns the ~1.3us/tile delivery
cadence. Groups process both 512-wide n-chunks back to back, relu
alternates vector/scalar, and linear matmuls flush in bursts of 12 (4-way
column-concurrent). Double-buffered row pools add WAR gating so row r+2's
transfer can't overwrite row r while in use. Outputs are cast to bf16 and
leave as two [42,1024] DMAs on separate queues.
"""
import numpy as np
import ml_dtypes

import concourse.mybir as mybir
import concourse.tile as tile
from concourse import bacc
from concourse.bass_utils import run_bass_kernel_spmd

BF16 = mybir.dt.bfloat16
F32 = mybir.dt.float32
RELU = mybir.ActivationFunctionType.Relu

B = 1024
IC, OC, NCLS = 3, 42, 10
KH = KW = 7
OH = OW = 26
NCORES = 8
N_ROWS = 4           # canonical band rows per core (3 full + 1 piece)
STRIDE = 22          # band rows per w'-block: 21 data + 1 ones(bias) row
WINLEN = 6 * STRIDE + 21   # partition span of one location window (153)
TPR = 6              # band tiles per canonical row (704 rows -> 6 tiles)
NB = 2               # two N-chunks of 512
NCHUNK = 512
N_WARM = 11          # PE warm-ups: gapless ramp until tiles t0/t1 are in

# Groups of consecutive locations within a canonical row: (w0, len)
GROUPS_FULL = [(0, 3), (3, 3), (6, 3), (9, 3), (12, 3), (15, 3), (18, 3),
               (21, 3), (24, 2)]
GROUPS_ROW3 = [(0, 3), (3, 3), (6, 2)]


def _group_tiles(w0, L):
    ta = (STRIDE * w0) // 128
    tb = (STRIDE * (w0 + L - 1) + WINLEN - 1) // 128
    return list(range(ta, tb + 1))


def _groups():
    """[(row, w0, L, [tiles], chunk0)] — chunk0 = first wt chunk index."""
    out = []
    ck = 0
    for r in range(N_ROWS):
        for w0, L in (GROUPS_FULL if r < 3 else GROUPS_ROW3):
            ts = _group_tiles(w0, L)
            out.append((r, w0, L, ts, ck))
            ck += len(ts)
    return out, ck

GROUPS, N_CHUNK_TOT = _groups()
NG = len(GROUPS)
# band tiles actually used per canonical row (row 3 only needs tiles 0-2)
ROW_TILES = [TPR, TPR, TPR, max(t for (r, _, _, ts, _) in GROUPS if r == 3
                                for t in ts) + 1]
# Weight transfer pieces (chunk ranges), each its own contiguous
# partition-major DRAM tensor: per-partition lines of (width*256)B, kept
# >=1.75KB so the line-rate-limited queue runs near full bandwidth. Row 0
# is split 8+7+7 so the first matmuls gate on 256KB.
WT_PIECES = [(0, 8), (8, 15), (15, 22), (22, 44), (44, 66), (66, 73)]

_cache = {}


def _build_program():
    if "nc" in _cache:
        return _cache["nc"]

    nc = bacc.Bacc("TRN2", target_bir_lowering=False, debug=False,
                   num_devices=NCORES)
    band_d = nc.dram_tensor("band", [N_ROWS * TPR, 128, B], BF16,
                            kind="ExternalInput").ap()
    wtp_d = [nc.dram_tensor(f"wtp{k}", [128, (b - a) * 128], BF16,
                            kind="ExternalInput").ap()
             for k, (a, b) in enumerate(WT_PIECES)]
    lwp_d = nc.dram_tensor("lwp", [128, NG * NCLS], BF16,
                           kind="ExternalInput").ap()
    # 2 slabs of PSUM col-tile partials (4 x 10-row slices at partition
    # offsets 0/32 within each slab), summed on host
    part_d = nc.dram_tensor("part", [2, 42, NB * NCHUNK], BF16,
                            kind="ExternalOutput").ap()

    row_groups = [[g for g in GROUPS if g[0] == r] for r in range(N_ROWS)]

    with tile.TileContext(nc) as tc:
        with (
            tc.tile_pool(name="sb", bufs=1) as sb,
            tc.tile_pool(name="band_pool", bufs=2) as band_pool,
            tc.tile_pool(name="wt_pool", bufs=2) as wt_pool,
            tc.tile_pool(name="stk_pool", bufs=22) as stk_pool,
            tc.tile_pool(name="pp_pool", bufs=6, space="PSUM") as pp_pool,
            tc.tile_pool(name="lin_pool", bufs=1, space="PSUM") as lin_pool,
        ):
            lwp_s = sb.tile([128, NG * NCLS], BF16)
            zz = sb.tile([128, NCHUNK], BF16)

            # PE warm-up: zero tile matmuls with no DMA deps keep the PE
            # busy (ramping the HW activity state) while band tile 0 and
            # weight piece 0 are in flight. They target lin_ps[0] (whose
            # real accumulation chains open with start=True much later) so
            # all 6 pp_pool buffers stay free for the runway below.
            nc.gpsimd.memset(zz, 0.0)

            WTW = max(gs[-1][4] + len(gs[-1][3]) - gs[0][4]
                      for gs in row_groups)        # chunks per row (<= 22)

            nc.gpsimd.dma_start(out=lwp_s, in_=lwp_d)

            def wt_piece(wtt, c0, k):
                a, b = WT_PIECES[k]
                nc.scalar.dma_start(out=wtt[:, (a - c0) * 128:
                                           (b - c0) * 128],
                                    in_=wtp_d[k])

            def start_row(r, interleave=False):
                nt = ROW_TILES[r]
                bt = band_pool.tile([128, TPR * B], BF16, tag="band")
                if interleave:
                    # Row 0's band tiles go per-tile on the sync queue in
                    # consumption order (per-tile granularity = earliest
                    # possible completion semaphores); weight pieces ride
                    # the scalar queue.
                    for t in range(nt):
                        nc.sync.dma_start(out=bt[:, t * B:(t + 1) * B],
                                          in_=band_d[r * TPR + t])
                    wtt0 = wt_pool.tile([128, WTW * 128], BF16, tag="wt")
                    for k in range(3):
                        wt_piece(wtt0, 0, k)
                    return bt, wtt0, 0
                else:
                    half = (nt + 1) // 2
                    nc.sync.dma_start(
                        out=bt[:, 0:half * B],
                        in_=band_d[r * TPR:r * TPR + half]
                        .transpose([1, 0, 2]))
                    nc.sync.dma_start(
                        out=bt[:, half * B:nt * B],
                        in_=band_d[r * TPR + half:r * TPR + nt]
                        .transpose([1, 0, 2]))
                gs = row_groups[r]
                c0 = gs[0][4]
                c1 = gs[-1][4] + len(gs[-1][3])
                wtt = wt_pool.tile([128, WTW * 128], BF16, tag="wt")
                for k, (a, b) in enumerate(WT_PIECES):
                    if c0 <= a < c1:
                        wt_piece(wtt, c0, k)
                return bt, wtt, c0

            # Linear layer: 4 PSUM column-tile slices per nb; groups are
            # assigned round-robin to col positions (0,32,64,96) and each
            # batch of linear matmuls is emitted back-to-back so they run
            # concurrently in disjoint PE column groups.
            lin_ps = [lin_pool.tile([128, NCHUNK], F32, name=f"lin_ps{nb}")
                      for nb in range(NB)]
            for i in range(N_WARM):
                nc.tensor.matmul(lin_ps[0], zz[:, 0:128], zz,
                                 start=True, stop=True,
                                 skip_group_check=True)
            # per (nb, pos): how many groups land there (for start/stop)
            npos = [[0] * 4 for _ in range(NB)]
            for k in range(NG):
                npos[0][k % 4] += 1
                npos[1][k % 4] += 1
            lin_done = [[0] * 4 for _ in range(NB)]

            def emit_lin(gi, nb, stk, k):
                pos = k % 4
                seen = lin_done[nb][pos]
                lin_done[nb][pos] += 1
                nc.tensor.matmul(
                    lin_ps[nb][32 * pos:32 * pos + NCLS, :],
                    lwp_s[:, gi * NCLS:(gi + 1) * NCLS],
                    stk,
                    start=(seen == 0), stop=(seen == npos[nb][pos] - 1),
                    tile_position=(0, 32 * pos),
                    skip_group_check=True,
                )

            relu_i = [0]

            def emit_relu(stk, pp):
                k = relu_i[0] % 2
                relu_i[0] += 1
                if k == 1:
                    nc.scalar.activation(stk, pp, RELU)
                else:
                    nc.vector.tensor_scalar_max(stk, pp, 0.0)

            pend = []
            lin_k = [0, 0]   # per-nb emitted-lin counter (drives col pos)

            def flush_lin(nmin, chunk=12):
                # bursts of 12 pipeline best: the 4 column positions cycle
                # three times and the col-tiled matmuls overlap ~4-deep
                while len(pend) >= nmin:
                    batch = [pend.pop(0)
                             for _ in range(min(chunk, len(pend)))]
                    for (gi, nb, stk) in batch:
                        emit_lin(gi, nb, stk, lin_k[nb])
                        lin_k[nb] += 1

            def finish_group(gi, pps):
                for nb in range(NB):
                    stk = stk_pool.tile([128, NCHUNK], BF16, tag="stk")
                    emit_relu(stk, pps[nb])
                    pend.append((gi, nb, stk))

            def runway(bt_s, wt_s):
                """Emit groups 0-2 of row 0 with their accumulation chains
                interleaved chunk-major: all tile-0 matmuls first, then
                tile-1, then tile-2 — so the PE (starting real matmuls
                while tiles are still landing every ~1.3us + semaphore
                lag) never outruns the DMA queue."""
                pps = [[pp_pool.tile([128, NCHUNK], F32, tag="pp",
                                     name=f"rw_pp{g}_{nb}")
                        for nb in range(NB)] for g in range(3)]

                def mm(g, cc, t, start, stop):
                    for nb in range(NB):
                        col = t * B + nb * NCHUNK
                        nc.tensor.matmul(
                            pps[g][nb],
                            wt_s[:, cc * 128:(cc + 1) * 128],
                            bt_s[:, col:col + NCHUNK],
                            start=start, stop=stop)
                mm(0, 0, 0, True, False)      # tile 0: c0, c2
                mm(1, 2, 0, True, False)
                mm(0, 1, 1, False, True)      # tile 1: c1, c3, c5
                mm(1, 3, 1, False, False)
                mm(2, 5, 1, True, False)
                finish_group(0, pps[0])
                mm(1, 4, 2, False, True)      # tile 2: c4, c6
                finish_group(1, pps[1])
                mm(2, 6, 2, False, True)
                finish_group(2, pps[2])

            cur = start_row(0, interleave=True)
            nxt = start_row(1)
            for r in range(N_ROWS):
                bt_s, wt_s, cbase = cur
                for (gr, w0, L, ts, ck) in row_groups[r]:
                    gi = GROUPS.index((gr, w0, L, ts, ck))
                    if r == 0 and gi == 0:
                        runway(bt_s, wt_s)
                        continue
                    if r == 0 and gi in (1, 2):
                        continue
                    pps = []
                    for nb in range(NB):
                        pp = pp_pool.tile([128, NCHUNK], F32, tag="pp")
                        pps.append(pp)
                        for ci, t in enumerate(ts):
                            cc = ck - cbase + ci
                            col = t * B + nb * NCHUNK
                            nc.tensor.matmul(
                                pp,
                                wt_s[:, cc * 128:(cc + 1) * 128],
                                bt_s[:, col:col + NCHUNK],
                                start=(ci == 0), stop=(ci == len(ts) - 1),
                            )
                    finish_group(gi, pps)
                    flush_lin(18)
                if r + 2 < N_ROWS:
                    cur, nxt = nxt, start_row(r + 2)
                else:
                    cur, nxt = nxt, None
            flush_lin(1)
            out_s = sb.tile([106, NB * NCHUNK], BF16)
            nc.vector.tensor_copy(out_s[:, 0:NCHUNK], lin_ps[0][0:106, :])
            nc.scalar.activation(out_s[:, NCHUNK:2 * NCHUNK],
                                 lin_ps[1][0:106, :],
                                 mybir.ActivationFunctionType.Copy)
            nc.sync.dma_start(out=part_d[0], in_=out_s[0:42, :],
                              single_packet=True)
            nc.scalar.dma_start(out=part_d[1], in_=out_s[64:106, :],
                                single_packet=True)

    nc.compile()
    _cache["nc"] = nc
    return nc


def _core_slots(c):
    """Actual (h, w) per canonical slot for core c; None = pad."""
    slots = []
    for i in range(78):
        slots.append((3 * c + i // 26, i % 26))
    p0 = (52 * c) // 8
    p1 = (52 * (c + 1)) // 8
    ph, pw0 = 24 + p0 // 26, p0 % 26
    plen = p1 - p0
    for j in range(8):
        slots.append((ph, pw0 + j) if j < plen else None)
    return slots, (ph, pw0, plen)


def _prep_core(c, x, W, b, lw4):
    """Build band / wt-piece / lwp arrays for core c."""
    slots, (ph, pw0, plen) = _core_slots(c)

    # bands ------------------------------------------------------------
    hs = [(3 * c, 0), (3 * c + 1, 0), (3 * c + 2, 0), (ph, pw0)]
    band = np.zeros((N_ROWS * TPR, 128, B), dtype=ml_dtypes.bfloat16)
    cj = np.arange(21) // 7          # channel per j
    kij = np.arange(21) % 7          # kernel-row per j
    for r, (h, shift) in enumerate(hs):
        nblocks = min(32, 32 - shift)
        wslice = np.arange(nblocks) + shift
        blk = x[:, cj[:, None], (h + kij)[:, None], wslice[None, :]]
        blk = blk.transpose(1, 2, 0)          # [21, nblocks, B]
        brow = np.zeros((TPR * 128, B), dtype=ml_dtypes.bfloat16)
        for bw in range(nblocks):
            brow[STRIDE * bw:STRIDE * bw + 21] = blk[:, bw]
            brow[STRIDE * bw + 21] = 1.0
        band[r * TPR:(r + 1) * TPR] = brow.reshape(TPR, 128, B)
    # wt ----------------------------------------------------------------
    wt = np.zeros((128, N_CHUNK_TOT * 128), dtype=ml_dtypes.bfloat16)
    for (r, w0, L, ts, ck) in GROUPS:
        for s in range(L):
            w_c = w0 + s
            sl = r * 26 + w_c if r < 3 else 78 + w_c
            hw = slots[sl]
            if hw is None:
                continue
            h, w = hw
            Wl = W[:, :, h, w, :]                 # [42, 3, 49]
            bl = b[:, h, w]                       # [42]
            for ci, t in enumerate(ts):
                col = (ck + ci) * 128 + 42 * s
                rel = 128 * t + np.arange(128) - STRIDE * w_c
                kj = rel // STRIDE
                jj = rel % STRIDE
                valid = (rel >= 0) & (rel < WINLEN) & (jj < 21)
                vals = np.zeros((128, OC), dtype=np.float32)
                vj, vk = jj[valid], kj[valid]
                vals[valid] = Wl[:, vj // 7, (vj % 7) * 7 + vk].T
                bias_row = (rel >= 0) & (rel < WINLEN) & (jj == 21) & (kj == 0)
                if bias_row.any():
                    vals[bias_row] = bl
                wt[:, col:col + OC] = vals.astype(ml_dtypes.bfloat16)
    # lwp ---------------------------------------------------------------
    lwp = np.zeros((128, NG * NCLS), dtype=ml_dtypes.bfloat16)
    for gi, (r, w0, L, ts, ck) in enumerate(GROUPS):
        for s in range(L):
            w_c = w0 + s
            sl = r * 26 + w_c if r < 3 else 78 + w_c
            if slots[sl] is None:
                continue
            h, w = slots[sl]
            lwp[42 * s:42 * s + OC, gi * NCLS:(gi + 1) * NCLS] = (
                lw4[:, :, h, w].T.astype(ml_dtypes.bfloat16)
            )
    out = {"band": band, "lwp": lwp}
    for k, (a, b_) in enumerate(WT_PIECES):
        out[f"wtp{k}"] = np.ascontiguousarray(wt[:, a * 128:b_ * 128])
    return out


def _run(x, W, b, lw, lb, trace=False):
    nc = _build_program()
    x = np.ascontiguousarray(np.asarray(x, dtype=np.float32))
    W = np.asarray(W, dtype=np.float32)
    b = np.asarray(b, dtype=np.float32)
    lw = np.asarray(lw, dtype=np.float32)
    lb = np.asarray(lb, dtype=np.float32)
    lw4 = lw.reshape(NCLS, OC, OH, OW)
    in_maps = [_prep_core(c, x, W, b, lw4) for c in range(NCORES)]
    res = run_bass_kernel_spmd(
        nc, in_maps, list(range(NCORES)), trace=trace,
    )
    part = np.zeros((2, 42, NB * NCHUNK), dtype=np.float32)
    for c in range(NCORES):
        part += np.asarray(res.results[c]["part"], dtype=np.float32)
    out10 = (part[0][0:NCLS] + part[0][32:32 + NCLS]
             + part[1][0:NCLS] + part[1][32:32 + NCLS])
    out = out10.T + lb[None, :]
    return out.astype(np.float32), res


def kernel(**inputs):
    out, _ = _run(inputs["x"], inputs["W"], inputs["b"], inputs["lw"],
                  inputs["lb"])
    return out
